# revision 6
# baseline (speedup 1.0000x reference)
"""Trainium2 Bass kernel for nn_CNN_ODE (CNN encoder + 50-step dopri5 neural ODE + regressor).

Strategy: pure data parallel over 8 NeuronCores (8192 samples/core), parameters
replicated. Per core, activations live feature-on-partition, two batch halves
stacked into 128 partitions ([128, 4096] tiles). The dopri5 step is reformulated
in "z-space" (z = W1 y): every linear combination of stage values becomes a
64x64 matmul with host-prescaled weights (V = W1@W2) accumulated in PSUM, so the
vector engine does almost nothing and the tensor engine runs 4 concurrent 64x64
quadrant matmuls (tile_position). tanh runs on the scalar engine at 128 lanes
with the per-stage bias folded in. fp16 operands / fp32 accumulation throughout
(validated: rel err ~3e-4 vs fp32 reference).

Layout bookkeeping: odd chunk-columns route through crossed PE quadrants, which
swap the two 64-partition halves; tanh outputs are swapped back by GpSimd
copies, the state update w += .. is done half-aware on DVE, and S is kept in two
step-parity accumulators that are merged (with one swap) before the regressor.
"""

import numpy as np

import concourse.bass as bass
import concourse.bacc as bacc
import concourse.mybir as mybir
from concourse.tile import TileContext
from concourse.bass_utils import run_bass_kernel_spmd

F16 = mybir.dt.float16
F32 = mybir.dt.float32
AF = mybir.ActivationFunctionType

N_CORES = 8
B_TOTAL = 65536
SEQ, IN_DIM, N_KER, KSZ = 40, 24, 36, 3
ENC_DIM, HID, REG = 128, 64, 32
ODE_STEPS = 50
# dopri5 tableau
_A = [
    [1 / 5],
    [3 / 40, 9 / 40],
    [44 / 45, -56 / 15, 32 / 9],
    [19372 / 6561, -25360 / 2187, 64448 / 6561, -212 / 729],
    [9017 / 3168, -355 / 33, 46732 / 5247, 49 / 176, -5103 / 18656],
]
_BW = [35 / 384, 0.0, 500 / 1113, 125 / 192, -2187 / 6784, 11 / 84]


def _ode_coef_lists(dt):
    """Returns (zchain_coefs(15 floats, emission order), ds_coefs(5 floats)).

    Chain covers stages 2..6 only (incremental differences). The step update
    z_{n+1} = z_n + V @ ds + gamma reuses the dS accumulator instead of a
    6-term tail, saving 5 matmul terms per step."""
    coef = np.zeros((7, 7))
    for i in range(2, 7):
        row = _A[i - 2]
        coef[i, 1 : 1 + len(row)] = np.array(row) * dt
    bw = np.array(_BW) * dt
    zc = []
    zc.append(coef[2, 1])
    for i in range(3, 7):
        for j in range(1, i):
            zc.append(coef[i, j] - coef[i - 1, j])
    ds = [bw[j - 1] for j in (1, 3, 4, 5, 6)]
    return zc, ds, coef, bw


def make_consts(inputs, steps=ODE_STEPS):
    """Host-side precompute of all device weight/bias tensors (fp64 math)."""
    f16 = np.float16
    g = {k: np.asarray(v, dtype=np.float64) for k, v in inputs.items() if k != "x"}
    dt = float(g["t_span"][1] - g["t_span"][0]) / steps
    W1, b1 = g["ode1_w"], g["ode1_b"]
    W2, b2 = g["ode2_w"], g["ode2_b"]
    V = W1 @ W2
    cvec = W1 @ b2
    zc, dsc, coef, bw = _ode_coef_lists(dt)

    c = {}
    # ---- ODE weights: [128, 21, 128] f16 block-diagonal (two sample halves)
    # idx 0..14 scaled V^T (chain stages 2..6), 15..19 scaled identities (dS),
    # 20 plain V^T (step update z += V @ ds + gamma)
    ow = np.zeros((128, 21, 128), np.float64)
    for idx, d in enumerate(zc):
        X = (d * V).T
        ow[0:64, idx, 0:64] = X
        ow[64:128, idx, 64:128] = X
    for k, d in enumerate(dsc):
        ow[:, 15 + k, :] = np.eye(128) * d
    VT = V.T
    ow[0:64, 20, 0:64] = VT
    ow[64:128, 20, 64:128] = VT
    c["ode_w"] = ow.astype(f16)
    beta = np.zeros((64, 6))
    beta[:, 0] = b1
    for i in range(2, 7):
        beta[:, i - 1] = b1 + coef[i].sum() * cvec
    c["beta"] = np.concatenate([beta, beta], axis=0).astype(np.float32)
    gam = (dt * cvec)[:, None]
    c["gamma"] = np.concatenate([gam, gam], axis=0).astype(np.float32)
    w1bd = np.zeros((128, 128))
    w1bd[0:64, 0:64] = W1.T
    w1bd[64:128, 64:128] = W1.T
    c["w1t"] = w1bd.astype(f16)

    # ---- conv lhsT blocks (c_out padded 36->64)
    cw = g["conv_w"]  # [36, 24, 3]

    def cv_block(n_si, so_count, k_of):
        # rows: (si, ci) over n_si x 24 from row 0; cols: 64*so + co
        out = np.zeros((24 * n_si, 64 * so_count), np.float64)
        for si in range(n_si):
            for ci in range(24):
                for so in range(so_count):
                    k = k_of(si, so)
                    if 0 <= k < 3:
                        out[24 * si + ci, 64 * so : 64 * so + 36] = cw[:, ci, k]
        return out

    # interior pair (4g+1, 4g+2), rhs rows 0..95 (si 0..3): k = si - so
    c["cv_int"] = cv_block(4, 2, lambda si, so: si - so).astype(f16)
    # cross a: rhs rows 0..95 (si<2 pad out as invalid-k): k = si - 2 - so
    c["cv_xa"] = cv_block(4, 2, lambda si, so: si - 2 - so).astype(f16)
    # cross b: chunk g+1 rows 0..47 (si' 0..1): k = si - so + 2
    c["cv_xb"] = cv_block(2, 2, lambda si, so: si - so + 2).astype(f16)
    # edge s0: rows 0..47 (si 0..1): k = si + 1
    c["cv_e0"] = cv_block(2, 1, lambda si, so: si + 1).astype(f16)
    # edge s39 + chunk-9 cross block: chunk 9 is transposed from col 832,
    # so its row u holds flat index 832+u -> s=(832+u)//24, c=(832+u)%24.
    e39 = np.zeros((128, 64))
    xb9 = np.zeros((128, 128))
    for u in range(128):
        s, ci = (832 + u) // 24, (832 + u) % 24
        if s in (38, 39):  # e39: k = s - 38
            e39[u, 0:36] = cw[:, ci, s - 38]
        if s in (36, 37):  # cross-b for pair (35,36): k = (s-36) - so + 2
            for so in range(2):
                k = (s - 36) - so + 2
                if 0 <= k < 3:
                    xb9[u, 64 * so : 64 * so + 36] = cw[:, ci, k]
    c["cv_e39"] = e39.astype(f16)
    c["cv_xb9"] = xb9.astype(f16)
    int9 = np.zeros((128, 128))
    for u in range(128):
        s, ci = (832 + u) // 24, (832 + u) % 24
        for so in range(2):
            k = s - (37 + so) + 1
            if 0 <= k < 3:
                int9[u, 64 * so : 64 * so + 36] = cw[:, ci, k]
    c["cv_int9"] = int9.astype(f16)
    cb = np.zeros((64, 1))
    cb[:36, 0] = g["conv_b"]
    c["conv_bias"] = np.concatenate([cb, cb], axis=0).astype(np.float32)

    # ---- enc1: [128, 20, 128] f16, blocks: 0 = edges(s0 rows0-63, s39 rows64-127),
    # j>=1: s = 2j-1 + r//64, co = r%64 ; flatten index co*40 + s
    e1w = g["enc1_w"]  # [128, 1440]
    e1 = np.zeros((128, 20, 128), np.float64)
    for j in range(20):
        for r in range(128):
            co = r % 64
            if co >= 36:
                continue
            s = (0 if r < 64 else 39) if j == 0 else (2 * j - 1 + r // 64)
            e1[r, j, :] = e1w[:, co * 40 + s]
    c["enc1_w"] = e1.astype(f16)
    c["enc1_bias"] = g["enc1_b"][:, None].astype(np.float32)  # [128,1]
    c["enc2_w"] = g["enc2_w"].T.astype(f16)  # [128, 64]
    c["enc2_bias"] = g["enc2_b"][:, None].astype(np.float32)  # [64,1]

    # ---- regressor
    R1, br1 = g["reg1_w"], g["reg1_b"]
    R2, br2 = g["reg2_w"], g["reg2_b"]
    r1ybd = np.zeros((128, 64))
    r1ybd[0:64, 0:32] = R1.T
    r1ybd[64:128, 32:64] = R1.T
    c["r1y"] = r1ybd.astype(f16)
    r1s = (R1 @ W2).T
    r1sbd = np.zeros((128, 64))
    r1sbd[0:64, 0:32] = r1s
    r1sbd[64:128, 32:64] = r1s
    c["r1s"] = r1sbd.astype(f16)
    bias_r = (R1 @ (steps * dt * b2) + br1)[:, None]
    c["bias_r"] = np.tile(bias_r, (4, 1)).astype(np.float32)  # [128,1]
    r2bd = np.zeros((128, 4))
    for b in range(4):
        r2bd[32 * b : 32 * b + 32, b] = R2[0]
    c["r2"] = r2bd.astype(f16)  # [128,4] block-diagonal
    c["br2"] = np.full((128, 1), br2[0], np.float32)
    return c


def _blob_layout():
    """Pack order + column offsets of consts inside the two dtype blobs."""
    off = {F16: 0, F32: 0}
    lay = {}
    for n, sh, dt in CONST_SPECS:
        cols = int(np.prod(sh[1:]))
        lay[n] = (dt, off[dt], cols, sh)
        off[dt] += cols
    return lay, off[F16], off[F32]


def pack_consts(c):
    lay, n16, n32 = _blob_layout()
    b16 = np.zeros((128, n16), np.float16)
    b32 = np.zeros((128, n32), np.float32)
    for n, (dt, off, cols, sh) in lay.items():
        arr = c[n].reshape(sh[0], cols)
        (b16 if dt == F16 else b32)[: sh[0], off : off + cols] = arr
    return b16, b32


CONST_SPECS = [
    ("ode_w", [128, 21, 128], F16),
    ("beta", [128, 6], F32),
    ("gamma", [128, 1], F32),
    ("w1t", [128, 128], F16),
    ("cv_int", [96, 128], F16),
    ("cv_xa", [96, 128], F16),
    ("cv_xb", [48, 128], F16),
    ("cv_e0", [48, 64], F16),
    ("cv_e39", [128, 64], F16),
    ("cv_xb9", [128, 128], F16),
    ("cv_int9", [128, 128], F16),
    ("conv_bias", [128, 1], F32),
    ("enc1_w", [128, 20, 128], F16),
    ("enc1_bias", [128, 1], F32),
    ("enc2_w", [128, 64], F16),
    ("enc2_bias", [64, 1], F32),
    ("r1y", [128, 64], F16),
    ("r1s", [128, 64], F16),
    ("bias_r", [128, 1], F32),
    ("r2", [128, 4], F16),
    ("br2", [128, 1], F32),
]


def build_nc(bpc, steps=ODE_STEPS, debug_tap=False):
    """Build the per-core Bass program (SPMD; identical on all cores)."""
    nc = bacc.Bacc("TRN2", target_bir_lowering=False)
    HB = bpc // 2            # stacked tile width (half-batch)
    NCH = HB // 512          # chunk-columns
    NW = HB // 1024          # ODE waves of 1024 cols
    NG = bpc // 512          # encoder groups

    x_in = nc.dram_tensor("x16t", [10, 128, bpc], F16, kind="ExternalInput")
    out_t = nc.dram_tensor("out", [bpc], F32, kind="ExternalOutput")
    dbg_t = (nc.dram_tensor("dbg", [128, bpc // 2], F32, kind="ExternalOutput")
             if debug_tap else None)
    lay, n16, n32 = _blob_layout()
    cb16_in = nc.dram_tensor("cb16", [128, n16], F16, kind="ExternalInput")
    cb32_in = nc.dram_tensor("cb32", [128, n32], F32, kind="ExternalInput")

    with TileContext(nc) as tc:
        import contextlib
        es = contextlib.ExitStack()
        with es:
            cpool = es.enter_context(tc.tile_pool(name="consts", bufs=1))
            big = es.enter_context(tc.tile_pool(name="big", bufs=1))

            # const tiles: two packed blobs -> sliced views
            cb16 = cpool.tile([128, n16], F16, tag="cb16", name="cb16")
            cb32 = cpool.tile([128, n32], F32, tag="cb32", name="cb32")
            nc.sync.dma_start(out=cb16[:], in_=cb16_in[:])
            nc.sync.dma_start(out=cb32[:], in_=cb32_in[:])
            ct = {}
            for n, (dt, off, cols, sh) in lay.items():
                v = (cb16 if dt == F16 else cb32)[: sh[0], off : off + cols]
                if len(sh) == 3:
                    v = v.rearrange("p (a b) -> p a b", b=sh[2])
                ct[n] = v

            # persistent state tiles
            w = big.tile([128, HB], F32, tag="w")
            S0 = big.tile([128, HB], F32, tag="S0")
            y0 = big.tile([128, HB], F16, tag="y0")
            tS = [big.tile([128, HB], F16, tag=f"t{i}", name=f"t{i}") for i in range(1, 7)]
            pred_sb = big.tile([128, HB // 2], F32, tag="pred")
            nc.gpsimd.memset(S0[:], 0.0)

            # ---------------- Phase 1: transpose + encoder ----------------

            def dest_of_group(g):
                # group g (512 samples) -> (row offset, chunk-col) in stacked tiles
                h, cc = (0, g) if g < NG // 2 else (1, g - NG // 2)
                return 64 * h, cc

            with tc.tile_pool(name="enc_sb", bufs=2) as epool, \
                 tc.tile_pool(name="enc_ps", bufs=3, space="PSUM") as cps, \
                 tc.tile_pool(name="enc_ps2", bufs=2, space="PSUM") as eps:
                for g in range(NG):
                    ro, cc = dest_of_group(g)
                    ccols = bass.ts(cc, 512)
                    xt = epool.tile([128, 10, 512], F16, tag="xt")
                    nc.sync.dma_start(
                        out=xt[:],
                        in_=x_in[:, :, g * 512 : (g + 1) * 512].rearrange(
                            "k p n -> p k n"),
                    )
                    h_t = epool.tile([128, 20, 512], F16, tag="h")
                    for pi in range(10):
                        cp = cps.tile([128, 1024], F32, tag="cps")
                        for hf in range(2):
                            b = 2 * pi + hf
                            pc = bass.ts(hf, 512)
                            if b == 0:
                                nc.tensor.matmul(
                                    cp[0:64, pc], ct["cv_e0"][:], xt[0:48, 0, :],
                                    start=True, stop=True, tile_position=(0, 0), skip_group_check=True)
                                nc.tensor.matmul(
                                    cp[64:128, pc], ct["cv_e39"][:], xt[:, 9, :],
                                    start=True, stop=True, tile_position=(0, 64), skip_group_check=True)
                            else:
                                s0 = 2 * b - 1
                                cg, pos = s0 // 4, s0 % 4
                                if pos == 1:
                                    lhs = "cv_int" if cg < 9 else "cv_int9"
                                    rhs = xt[0:96, cg, :] if cg < 9 else xt[:, 9, :]
                                    nc.tensor.matmul(
                                        cp[:, pc], ct[lhs][:], rhs,
                                        start=True, stop=True, skip_group_check=True)
                                else:  # pos == 3, cross
                                    nc.tensor.matmul(
                                        cp[:, pc], ct["cv_xa"][:], xt[0:96, cg, :],
                                        start=True, stop=False, skip_group_check=True)
                                    if cg + 1 < 9:
                                        nc.tensor.matmul(
                                            cp[:, pc], ct["cv_xb"][:],
                                            xt[0:48, cg + 1, :],
                                            start=False, stop=True, skip_group_check=True)
                                    else:
                                        nc.tensor.matmul(
                                            cp[:, pc], ct["cv_xb9"][:],
                                            xt[:, 9, :],
                                            start=False, stop=True, skip_group_check=True)
                        sg = epool.tile([128, 1024], F16, tag="sg")
                        nc.scalar.activation(sg[:], cp[:], AF.Sigmoid,
                                             bias=ct["conv_bias"][:])
                        nc.vector.scalar_tensor_tensor(
                            out=h_t[:, 2 * pi : 2 * pi + 2, :].rearrange(
                                "p a b -> p (a b)"),
                            in0=cp[:], scalar=ct["conv_bias"][:], in1=sg[:],
                            op0=mybir.AluOpType.add, op1=mybir.AluOpType.mult)
                    ep = eps.tile([128, 512], F32, tag="ep")
                    for j in range(20):
                        nc.tensor.matmul(ep[:], ct["enc1_w"][:, j, :], h_t[:, j, :],
                                         start=(j == 0), stop=(j == 19), skip_group_check=True)
                    e1 = epool.tile([128, 512], F16, tag="e1")
                    nc.scalar.activation(e1[:], ep[:], AF.Relu,
                                         bias=ct["enc1_bias"][:])
                    tp = eps.tile([128, 512], F32, tag="ep")
                    nc.tensor.matmul(tp[0:64, :], ct["enc2_w"][:], e1[:],
                                     start=True, stop=True, skip_group_check=True)
                    nc.scalar.activation(y0[ro : ro + 64, ccols], tp[0:64, :],
                                         AF.Identity, bias=ct["enc2_bias"][:])

                # w0 = W1 @ y0 (block-diagonal over sample halves)
                for cc in range(NCH):
                    ccols = bass.ts(cc, 512)
                    wp = eps.tile([128, 512], F32, tag="ep")
                    nc.tensor.matmul(wp[:], ct["w1t"][:], y0[:, ccols],
                                     start=True, stop=True, skip_group_check=True)
                    nc.vector.tensor_copy(out=w[:, ccols], in_=wp[:])

            if dbg_t is not None:
                dbg_sb = big.tile([128, HB], F32, tag="dbgsb")
                nc.vector.tensor_copy(out=dbg_sb[:], in_=y0[:])
                nc.sync.dma_start(out=dbg_t[:], in_=dbg_sb[:])

            # ---------------- Phase 2: ODE ----------------
            MMCH = 512  # matmul moving-operand chunk (ISA max 512 elements)

            def mm2(ps, lidx, rhs, start, stop):
                """One term: K=128 block-diagonal matmul(s) over a 1024-col
                wave; rhs is pre-sliced [128, 1024]."""
                lw = ct["ode_w"]
                for c0 in range(0, 1024, MMCH):
                    nc.tensor.matmul(ps[:, c0 : c0 + MMCH],
                                     lw[:, lidx, :], rhs[:, c0 : c0 + MMCH],
                                     start=start, stop=stop,
                                     skip_group_check=True)

            with tc.tile_pool(name="ode_ps", bufs=2, space="PSUM") as zpool, \
                 tc.tile_pool(name="ds_ps", bufs=2, space="PSUM") as dpool, \
                 tc.tile_pool(name="ode_sb", bufs=4) as opool:
                for n in range(steps):
                    for v in range(NW):
                        vc = bass.ts(v, 1024)
                        zb = zpool.tile([128, 1024], F32, tag="zb")
                        ds = dpool.tile([128, 1024], F32, tag="ds")
                        # t1 = tanh(w + b1)
                        nc.scalar.activation(tS[0][:, vc], w[:, vc], AF.Tanh,
                                             bias=ct["beta"][:, 0:1])
                        # chain: term (2,1) clears banks, then add w via DVE
                        mm2(zb, 0, tS[0][:, vc], True, False)
                        nc.vector.tensor_add(out=zb[:], in0=zb[:], in1=w[:, vc])
                        # dS term j=1 as soon as t1 exists
                        mm2(ds, 15, tS[0][:, vc], True, False)
                        li = 1
                        for i in range(3, 7):  # tanh t_{i-1}; stage-i diffs
                            ti = tS[i - 2]
                            nc.scalar.activation(ti[:, vc], zb[:], AF.Tanh,
                                                 bias=ct["beta"][:, i - 2 : i - 1])
                            for j in range(1, i):
                                last = (i == 6) and (j == i - 1)
                                mm2(zb, li, tS[j - 1][:, vc], False, last)
                                li += 1
                            if i >= 4:  # dS term j=i-1 (j in 3,4,5)
                                mm2(ds, 15 + (i - 3), tS[i - 2][:, vc],
                                    False, False)
                        nc.scalar.activation(tS[5][:, vc], zb[:], AF.Tanh,
                                             bias=ct["beta"][:, 5:6])
                        mm2(ds, 19, tS[5][:, vc], False, True)
                        # S += ds; dsq = f16(ds) on ACT; PSUM tile then
                        # reused for vds = V @ dsq; w += vds + gamma
                        nc.vector.tensor_add(out=S0[:, vc], in0=S0[:, vc],
                                             in1=ds[:])
                        dsq = opool.tile([128, 1024], F16, tag="dsq")
                        nc.scalar.activation(dsq[:], ds[:], AF.Identity)
                        mm2(ds, 20, dsq[:], True, True)
                        nc.vector.scalar_tensor_tensor(
                            out=w[:, vc], in0=ds[:], scalar=ct["gamma"][:],
                            in1=w[:, vc], op0=mybir.AluOpType.add,
                            op1=mybir.AluOpType.add)

                # ---------------- Phase 3: regressor ----------------
                S16 = tS[0]  # reuse t1 tile as f16 S
                nc.vector.tensor_copy(out=S16[:], in_=S0[:])

                for pr in range(NCH // 2):
                    rp = zpool.tile([128, 1024], F32, tag="zb")
                    for idx in range(2):
                        cc = 2 * pr + idx
                        ccols = bass.ts(cc, 512)
                        orow = slice(64 * idx, 64 * idx + 64)
                        tp_ = (0, 64 * idx)
                        nc.tensor.matmul(rp[orow, 0:512], ct["r1y"][:],
                                         y0[:, ccols], start=True, stop=False,
                                         tile_position=tp_, skip_group_check=True)
                        nc.tensor.matmul(rp[orow, 0:512], ct["r1s"][:],
                                         S16[:, ccols], start=False, stop=True,
                                         tile_position=tp_, skip_group_check=True)
                    rr = opool.tile([128, 512], F16, tag="rr")
                    nc.scalar.activation(rr[:], rp[:, 0:512], AF.Relu,
                                         bias=ct["bias_r"][:])
                    pp = dpool.tile([128, 1024], F32, tag="ds")
                    nc.tensor.matmul(pp[0:4, 0:512], ct["r2"][:], rr[:],
                                     start=True, stop=True,
                                     skip_group_check=True)
                    nc.vector.tensor_scalar_add(out=pred_sb[0:4, bass.ts(pr, 512)],
                                                in0=pp[0:4, 0:512],
                                                scalar1=ct["br2"][0:4])

                # out DMA: pred_sb[32*k, pr, n] -> sample mapping
                pv = pred_sb.rearrange("p (q n) -> p q n", n=512)
                ov = out_t.rearrange("(h q par n) -> h par q n", h=2, par=2, n=512)
                npair = NCH // 2
                # rows 0: (h0, even cc), 32: (h1, even), 64: (h0, odd), 96: (h1, odd)
                for k, (h, par) in enumerate([(0, 0), (1, 0), (0, 1), (1, 1)]):
                    nc.sync.dma_start(
                        out=ov[h, par],
                        in_=pv[k : k + 1, 0:npair, :],
                    )
    nc.compile()
    return nc


_CACHE = {}


def _get_nc(bpc, steps):
    key = (bpc, steps)
    if key not in _CACHE:
        _CACHE[key] = build_nc(bpc, steps)
    return _CACHE[key]


def make_in_maps(inputs):
    x = np.asarray(inputs["x"])
    bpc = x.shape[0] // N_CORES
    x16 = x.reshape(x.shape[0], SEQ * IN_DIM).astype(np.float16)
    # host-side transpose into the conv chunk layout: chunk k holds flat
    # feature rows off(k)..off(k)+127 (s-major (s,c)), samples along free dim
    x16t = np.stack([x16[:, (96 * k if k < 9 else 832):
                          (96 * k if k < 9 else 832) + 128].T
                     for k in range(10)])  # [10, 128, B]
    consts = make_consts(inputs)
    b16, b32 = pack_consts(consts)
    base = {"cb16": b16, "cb32": b32}
    return bpc, [dict(base,
                      x16t=np.ascontiguousarray(x16t[:, :, i * bpc:(i + 1) * bpc]))
                 for i in range(N_CORES)]


def kernel(**inputs):
    bpc, in_maps = make_in_maps(inputs)
    nc = _get_nc(bpc, ODE_STEPS)
    res = run_bass_kernel_spmd(nc, in_maps, list(range(N_CORES)))
    return np.concatenate([res.results[i]["out"] for i in range(N_CORES)])



# revision 16
# speedup vs baseline: 1.0710x; 1.0710x over previous
"""Trainium2 Bass kernel for nn_CNN_ODE (CNN encoder + 50-step dopri5 neural ODE + regressor).

Strategy: pure data parallel over 8 NeuronCores (8192 samples/core), parameters
replicated. Per core, activations live feature-on-partition, two batch halves
stacked into 128 partitions ([128, 4096] tiles). The dopri5 step is reformulated
in "z-space" (z = W1 y): every linear combination of stage values becomes a
64x64 matmul with host-prescaled weights (V = W1@W2) accumulated in PSUM, so the
vector engine does almost nothing and the tensor engine runs 4 concurrent 64x64
quadrant matmuls (tile_position). tanh runs on the scalar engine at 128 lanes
with the per-stage bias folded in. fp16 operands / fp32 accumulation throughout
(validated: rel err ~3e-4 vs fp32 reference).

Layout bookkeeping: odd chunk-columns route through crossed PE quadrants, which
swap the two 64-partition halves; tanh outputs are swapped back by GpSimd
copies, the state update w += .. is done half-aware on DVE, and S is kept in two
step-parity accumulators that are merged (with one swap) before the regressor.
"""

import numpy as np

import concourse.bass as bass
import concourse.bacc as bacc
import concourse.mybir as mybir
from concourse.tile import TileContext
from concourse.bass_utils import run_bass_kernel_spmd

F8 = mybir.dt.float8e4
F16 = mybir.dt.float16
F32 = mybir.dt.float32
AF = mybir.ActivationFunctionType

SC = 4096.0  # chain-weight fp8 normalization scale (zb holds SC*z)
DS_SC = 32.0  # dS identity-weight scale (ds PSUM holds DS_SC*ds)

N_CORES = 8
B_TOTAL = 65536
SEQ, IN_DIM, N_KER, KSZ = 40, 24, 36, 3
ENC_DIM, HID, REG = 128, 64, 32
ODE_STEPS = 50
# dopri5 tableau
_A = [
    [1 / 5],
    [3 / 40, 9 / 40],
    [44 / 45, -56 / 15, 32 / 9],
    [19372 / 6561, -25360 / 2187, 64448 / 6561, -212 / 729],
    [9017 / 3168, -355 / 33, 46732 / 5247, 49 / 176, -5103 / 18656],
]
_BW = [35 / 384, 0.0, 500 / 1113, 125 / 192, -2187 / 6784, 11 / 84]


def _ode_coef_lists(dt):
    """Returns (zchain_coefs(15 floats, emission order), ds_coefs(5 floats)).

    Chain covers stages 2..6 only (incremental differences). The step update
    z_{n+1} = z_n + V @ ds + gamma reuses the dS accumulator instead of a
    6-term tail, saving 5 matmul terms per step."""
    coef = np.zeros((7, 7))
    for i in range(2, 7):
        row = _A[i - 2]
        coef[i, 1 : 1 + len(row)] = np.array(row) * dt
    bw = np.array(_BW) * dt
    zc = []
    zc.append(coef[2, 1])
    for i in range(3, 7):
        for j in range(1, i):
            zc.append(coef[i, j] - coef[i - 1, j])
    ds = [bw[j - 1] for j in (1, 3, 4, 5, 6)]
    return zc, ds, coef, bw


def make_consts(inputs, steps=ODE_STEPS):
    """Host-side precompute of all device weight/bias tensors (fp64 math)."""
    f16 = np.float16
    g = {k: np.asarray(v, dtype=np.float64) for k, v in inputs.items() if k != "x"}
    dt = float(g["t_span"][1] - g["t_span"][0]) / steps
    W1, b1 = g["ode1_w"], g["ode1_b"]
    W2, b2 = g["ode2_w"], g["ode2_b"]
    V = W1 @ W2
    cvec = W1 @ b2
    zc, dsc, coef, bw = _ode_coef_lists(dt)

    c = {}
    f8 = mybir.dt.np(F8)
    # ---- ODE pair weights: [128, 13, 2, 128] fp8 block-diagonal DoubleRow
    # matmuls. Emission order (slotA, slotB; slot7 = zeros):
    #  0 s2 (t1,-) | 1 s3 (t1,t2) | 2 s4 (t1,t2) | 3 s4 (t3,-)
    #  4 s5 (t1,t2) | 5 s5 (t3,t4) | 6 s6 (t1,t2) | 7 s6 (t3,t4) | 8 s6 (t5,-)
    #  9 dS (t1,t3) | 10 dS (t4,t5) | 11 dS (t6,-) | 12 vds (dsq,-)
    zcm = [d * V for d in zc]
    pairs = [
        (SC * zcm[0], None), (SC * zcm[1], SC * zcm[2]),
        (SC * zcm[3], SC * zcm[4]), (SC * zcm[5], None),
        (SC * zcm[6], SC * zcm[7]), (SC * zcm[8], SC * zcm[9]),
        (SC * zcm[10], SC * zcm[11]), (SC * zcm[12], SC * zcm[13]),
        (SC * zcm[14], None),
    ]
    E = np.eye(64)
    pairs += [
        (DS_SC * dsc[0] * E, DS_SC * dsc[1] * E),
        (DS_SC * dsc[2] * E, DS_SC * dsc[3] * E),
        (DS_SC * dsc[4] * E, None),
        (V, None),
    ]
    ow8 = np.zeros((128, 13, 2, 128))
    for idx, (a, b) in enumerate(pairs):
        for o, X in ((0, a), (1, b)):
            if X is not None:
                ow8[0:64, idx, o, 0:64] = X.T
                ow8[64:128, idx, o, 64:128] = X.T
    c["ode8_w"] = ow8.astype(f8)
    # per-step biases with gamma-deficit correction: beta_n[i] = beta_i + n*g
    beta = np.zeros((64, 6))
    beta[:, 0] = b1
    for i in range(2, 7):
        beta[:, i - 1] = b1 + coef[i].sum() * cvec
    gam = dt * cvec
    bsteps = beta[:, None, :] + np.arange(steps)[None, :, None] * gam[:, None, None]
    bsteps = bsteps.reshape(64, steps * 6)
    c["beta"] = np.concatenate([bsteps, bsteps], axis=0).astype(np.float32)
    w1bd = np.zeros((128, 128))
    w1bd[0:64, 0:64] = W1.T
    w1bd[64:128, 64:128] = W1.T
    c["w1t"] = w1bd.astype(f16)

    # ---- conv lhsT blocks (c_out padded 36->64)
    cw = g["conv_w"]  # [36, 24, 3]

    def cv_block(n_si, so_count, k_of):
        # rows: (si, ci) over n_si x 24 from row 0; cols: 64*so + co
        out = np.zeros((24 * n_si, 64 * so_count), np.float64)
        for si in range(n_si):
            for ci in range(24):
                for so in range(so_count):
                    k = k_of(si, so)
                    if 0 <= k < 3:
                        out[24 * si + ci, 64 * so : 64 * so + 36] = cw[:, ci, k]
        return out

    # interior pair (4g+1, 4g+2), rhs rows 0..95 (si 0..3): k = si - so
    c["cv_int"] = cv_block(4, 2, lambda si, so: si - so).astype(f16)
    # cross a: rhs rows 0..95 (si<2 pad out as invalid-k): k = si - 2 - so
    c["cv_xa"] = cv_block(4, 2, lambda si, so: si - 2 - so).astype(f16)
    # cross b: chunk g+1 rows 0..47 (si' 0..1): k = si - so + 2
    c["cv_xb"] = cv_block(2, 2, lambda si, so: si - so + 2).astype(f16)
    # edge s0: rows 0..47 (si 0..1): k = si + 1
    c["cv_e0"] = cv_block(2, 1, lambda si, so: si + 1).astype(f16)
    # edge s39 + chunk-9 cross block: chunk 9 is transposed from col 832,
    # so its row u holds flat index 832+u -> s=(832+u)//24, c=(832+u)%24.
    e39 = np.zeros((128, 64))
    xb9 = np.zeros((128, 128))
    for u in range(128):
        s, ci = (832 + u) // 24, (832 + u) % 24
        if s in (38, 39):  # e39: k = s - 38
            e39[u, 0:36] = cw[:, ci, s - 38]
        if s in (36, 37):  # cross-b for pair (35,36): k = (s-36) - so + 2
            for so in range(2):
                k = (s - 36) - so + 2
                if 0 <= k < 3:
                    xb9[u, 64 * so : 64 * so + 36] = cw[:, ci, k]
    c["cv_e39"] = e39.astype(f16)
    c["cv_xb9"] = xb9.astype(f16)
    int9 = np.zeros((128, 128))
    for u in range(128):
        s, ci = (832 + u) // 24, (832 + u) % 24
        for so in range(2):
            k = s - (37 + so) + 1
            if 0 <= k < 3:
                int9[u, 64 * so : 64 * so + 36] = cw[:, ci, k]
    c["cv_int9"] = int9.astype(f16)
    cb = np.zeros((64, 1))
    cb[:36, 0] = g["conv_b"]
    c["conv_bias"] = np.concatenate([cb, cb], axis=0).astype(np.float32)

    # ---- enc1: [128, 20, 128] f16, blocks: 0 = edges(s0 rows0-63, s39 rows64-127),
    # j>=1: s = 2j-1 + r//64, co = r%64 ; flatten index co*40 + s
    e1w = g["enc1_w"]  # [128, 1440]
    e1 = np.zeros((128, 20, 128), np.float64)
    for j in range(20):
        for r in range(128):
            co = r % 64
            if co >= 36:
                continue
            s = (0 if r < 64 else 39) if j == 0 else (2 * j - 1 + r // 64)
            e1[r, j, :] = e1w[:, co * 40 + s]
    c["enc1_w"] = e1.astype(f16)
    c["enc1_bias"] = g["enc1_b"][:, None].astype(np.float32)  # [128,1]
    c["enc2_w"] = g["enc2_w"].T.astype(f16)  # [128, 64]
    c["enc2_bias"] = g["enc2_b"][:, None].astype(np.float32)  # [64,1]

    # ---- regressor
    R1, br1 = g["reg1_w"], g["reg1_b"]
    R2, br2 = g["reg2_w"], g["reg2_b"]
    r1ybd = np.zeros((128, 64))
    r1ybd[0:64, 0:32] = R1.T
    r1ybd[64:128, 32:64] = R1.T
    c["r1y"] = r1ybd.astype(f16)
    r1s = (R1 @ W2).T
    r1sbd = np.zeros((128, 64))
    r1sbd[0:64, 0:32] = r1s
    r1sbd[64:128, 32:64] = r1s
    c["r1s"] = r1sbd.astype(f16)
    bias_r = (R1 @ (steps * dt * b2) + br1)[:, None]
    c["bias_r"] = np.tile(bias_r, (4, 1)).astype(np.float32)  # [128,1]
    r2bd = np.zeros((128, 4))
    for b in range(4):
        r2bd[32 * b : 32 * b + 32, b] = R2[0]
    c["r2"] = r2bd.astype(f16)  # [128,4] block-diagonal
    c["br2"] = np.full((128, 1), br2[0], np.float32)
    return c


def _blob_layout():
    """Pack order + column offsets of consts inside the three dtype blobs."""
    off = {F16: 0, F32: 0, F8: 0}
    lay = {}
    for n, sh, dt in CONST_SPECS:
        cols = int(np.prod(sh[1:]))
        lay[n] = (dt, off[dt], cols, sh)
        off[dt] += cols
    return lay, off[F16], off[F32], off[F8]


def pack_consts(c):
    lay, n16, n32, n8 = _blob_layout()
    b16 = np.zeros((128, n16), np.float16)
    b32 = np.zeros((128, n32), np.float32)
    b8 = np.zeros((128, n8), mybir.dt.np(F8))
    blobs = {F16: b16, F32: b32, F8: b8}
    for n, (dt, off, cols, sh) in lay.items():
        arr = c[n].reshape(sh[0], cols)
        blobs[dt][: sh[0], off : off + cols] = arr
    return b16, b32, b8


CONST_SPECS = [
    ("ode8_w", [128, 13, 2, 128], F8),
    ("beta", [128, ODE_STEPS * 6], F32),
    ("w1t", [128, 128], F16),
    ("cv_int", [96, 128], F16),
    ("cv_xa", [96, 128], F16),
    ("cv_xb", [48, 128], F16),
    ("cv_e0", [48, 64], F16),
    ("cv_e39", [128, 64], F16),
    ("cv_xb9", [128, 128], F16),
    ("cv_int9", [128, 128], F16),
    ("conv_bias", [128, 1], F32),
    ("enc1_w", [128, 20, 128], F16),
    ("enc1_bias", [128, 1], F32),
    ("enc2_w", [128, 64], F16),
    ("enc2_bias", [64, 1], F32),
    ("r1y", [128, 64], F16),
    ("r1s", [128, 64], F16),
    ("bias_r", [128, 1], F32),
    ("r2", [128, 4], F16),
    ("br2", [128, 1], F32),
]


def build_nc(bpc, steps=ODE_STEPS, debug_tap=False):
    """Build the per-core Bass program (SPMD; identical on all cores)."""
    nc = bacc.Bacc("TRN2", target_bir_lowering=False)
    HB = bpc // 2            # stacked tile width (half-batch)
    NCH = HB // 512          # chunk-columns
    NW = HB // 1024          # ODE waves of 1024 cols
    NG = bpc // 512          # encoder groups

    x_in = nc.dram_tensor("x16t", [10, 128, bpc], F16, kind="ExternalInput")
    out_t = nc.dram_tensor("out", [bpc], F32, kind="ExternalOutput")
    dbg_t = (nc.dram_tensor("dbg", [128, bpc // 2], F32, kind="ExternalOutput")
             if debug_tap else None)
    lay, n16, n32, n8 = _blob_layout()
    cb16_in = nc.dram_tensor("cb16", [128, n16], F16, kind="ExternalInput")
    cb32_in = nc.dram_tensor("cb32", [128, n32], F32, kind="ExternalInput")
    cb8_in = nc.dram_tensor("cb8", [128, n8], F8, kind="ExternalInput")

    with TileContext(nc) as tc:
        import contextlib
        es = contextlib.ExitStack()
        with es:
            cpool = es.enter_context(tc.tile_pool(name="consts", bufs=1))
            big = es.enter_context(tc.tile_pool(name="big", bufs=1))

            # const tiles: three packed blobs -> sliced views
            cb16 = cpool.tile([128, n16], F16, tag="cb16", name="cb16")
            cb32 = cpool.tile([128, n32], F32, tag="cb32", name="cb32")
            cb8 = cpool.tile([128, n8], F8, tag="cb8", name="cb8")
            nc.sync.dma_start(out=cb16[:], in_=cb16_in[:])
            nc.sync.dma_start(out=cb32[:], in_=cb32_in[:])
            nc.sync.dma_start(out=cb8[:], in_=cb8_in[:])
            blobs = {F16: cb16, F32: cb32, F8: cb8}
            ct = {}
            for n, (dt, off, cols, sh) in lay.items():
                v = blobs[dt][: sh[0], off : off + cols]
                if len(sh) == 3:
                    v = v.rearrange("p (a b) -> p a b", b=sh[2])
                elif len(sh) == 4:
                    v = v.rearrange("p (a b c) -> p a b c", b=sh[2], c=sh[3])
                ct[n] = v

            # persistent state tiles
            w = big.tile([128, HB], F32, tag="w")
            S0 = big.tile([128, HB], F32, tag="S0")
            y0 = big.tile([128, HB], F16, tag="y0")
            # fp8 slot tile: 0..5 = t1..t6, 6 = dsq, 7 = zeros (pair filler)
            Tt = big.tile([128, 8, HB], F8, tag="Tt")
            sreg = big.tile([128, HB], F16, tag="sreg")
            pred_sb = big.tile([128, HB // 2], F32, tag="pred")
            nc.gpsimd.memset(S0[:], 0.0)
            nc.gpsimd.memset(Tt[:, 7, :], 0.0)

            # ---------------- Phase 1: transpose + encoder ----------------

            def dest_of_group(g):
                # group g (512 samples) -> (row offset, chunk-col) in stacked tiles
                h, cc = (0, g) if g < NG // 2 else (1, g - NG // 2)
                return 64 * h, cc

            with tc.tile_pool(name="enc_sb", bufs=2) as epool, \
                 tc.tile_pool(name="enc_ps", bufs=3, space="PSUM") as cps, \
                 tc.tile_pool(name="enc_ps2", bufs=2, space="PSUM") as eps:
                for g in range(NG):
                    ro, cc = dest_of_group(g)
                    ccols = bass.ts(cc, 512)
                    xt = epool.tile([128, 10, 512], F16, tag="xt")
                    nc.sync.dma_start(
                        out=xt[:],
                        in_=x_in[:, :, g * 512 : (g + 1) * 512].rearrange(
                            "k p n -> p k n"),
                    )
                    h_t = epool.tile([128, 20, 512], F16, tag="h")
                    for pi in range(10):
                        cp = cps.tile([128, 1024], F32, tag="cps")
                        for hf in range(2):
                            b = 2 * pi + hf
                            pc = bass.ts(hf, 512)
                            if b == 0:
                                nc.tensor.matmul(
                                    cp[0:64, pc], ct["cv_e0"][:], xt[0:48, 0, :],
                                    start=True, stop=True, tile_position=(0, 0), skip_group_check=True)
                                nc.tensor.matmul(
                                    cp[64:128, pc], ct["cv_e39"][:], xt[:, 9, :],
                                    start=True, stop=True, tile_position=(0, 64), skip_group_check=True)
                            else:
                                s0 = 2 * b - 1
                                cg, pos = s0 // 4, s0 % 4
                                if pos == 1:
                                    lhs = "cv_int" if cg < 9 else "cv_int9"
                                    rhs = xt[0:96, cg, :] if cg < 9 else xt[:, 9, :]
                                    nc.tensor.matmul(
                                        cp[:, pc], ct[lhs][:], rhs,
                                        start=True, stop=True, skip_group_check=True)
                                else:  # pos == 3, cross
                                    nc.tensor.matmul(
                                        cp[:, pc], ct["cv_xa"][:], xt[0:96, cg, :],
                                        start=True, stop=False, skip_group_check=True)
                                    if cg + 1 < 9:
                                        nc.tensor.matmul(
                                            cp[:, pc], ct["cv_xb"][:],
                                            xt[0:48, cg + 1, :],
                                            start=False, stop=True, skip_group_check=True)
                                    else:
                                        nc.tensor.matmul(
                                            cp[:, pc], ct["cv_xb9"][:],
                                            xt[:, 9, :],
                                            start=False, stop=True, skip_group_check=True)
                        sg = epool.tile([128, 1024], F16, tag="sg")
                        nc.scalar.activation(sg[:], cp[:], AF.Sigmoid,
                                             bias=ct["conv_bias"][:])
                        nc.vector.scalar_tensor_tensor(
                            out=h_t[:, 2 * pi : 2 * pi + 2, :].rearrange(
                                "p a b -> p (a b)"),
                            in0=cp[:], scalar=ct["conv_bias"][:], in1=sg[:],
                            op0=mybir.AluOpType.add, op1=mybir.AluOpType.mult)
                    ep = eps.tile([128, 512], F32, tag="ep")
                    for j in range(20):
                        nc.tensor.matmul(ep[:], ct["enc1_w"][:, j, :], h_t[:, j, :],
                                         start=(j == 0), stop=(j == 19), skip_group_check=True)
                    e1 = epool.tile([128, 512], F16, tag="e1")
                    nc.scalar.activation(e1[:], ep[:], AF.Relu,
                                         bias=ct["enc1_bias"][:])
                    tp = eps.tile([128, 512], F32, tag="ep")
                    nc.tensor.matmul(tp[0:64, :], ct["enc2_w"][:], e1[:],
                                     start=True, stop=True, skip_group_check=True)
                    nc.scalar.activation(y0[ro : ro + 64, ccols], tp[0:64, :],
                                         AF.Identity, bias=ct["enc2_bias"][:])

                # w0 = W1 @ y0 (block-diagonal over sample halves)
                for cc in range(NCH):
                    ccols = bass.ts(cc, 512)
                    wp = eps.tile([128, 512], F32, tag="ep")
                    nc.tensor.matmul(wp[:], ct["w1t"][:], y0[:, ccols],
                                     start=True, stop=True, skip_group_check=True)
                    nc.vector.tensor_copy(out=w[:, ccols], in_=wp[:])

            if dbg_t is not None:
                dbg_sb = big.tile([128, HB], F32, tag="dbgsb")
                nc.vector.tensor_copy(out=dbg_sb[:], in_=y0[:])
                nc.sync.dma_start(out=dbg_t[:], in_=dbg_sb[:])

            # ---------------- Phase 2: ODE (fp8 DoubleRow pairs) ----------
            pstride = Tt.ap[0][0]

            def pair_rhs(a, b, col0, ncols):
                """[128, 2, ncols] fp8 view pairing slots a and b of Tt."""
                return bass.AP(
                    Tt.tensor, Tt.offset + a * HB + col0,
                    [[pstride, 128], [(b - a) * HB, 2], [1, ncols]],
                )

            def mm8(ps, lidx, a, b, v, start, stop):
                for c0 in (0, 512):
                    nc.tensor.matmul(
                        ps[:, c0 : c0 + 512], ct["ode8_w"][:, lidx, :, :],
                        pair_rhs(a, b, v * 1024 + c0, 512),
                        start=start, stop=stop,
                        perf_mode=mybir.MatmulPerfMode.DoubleRow,
                        skip_group_check=True)

            # chain mms per stage i (lidx, slotA, slotB)
            STAGE_MMS = {
                3: [(1, 0, 1)],
                4: [(2, 0, 1), (3, 2, 7)],
                5: [(4, 0, 1), (5, 2, 3)],
                6: [(6, 0, 1), (7, 2, 3), (8, 4, 7)],
            }
            MULT = mybir.AluOpType.mult
            ADD = mybir.AluOpType.add

            with tc.tile_pool(name="ode_ps", bufs=2, space="PSUM") as zpool, \
                 tc.tile_pool(name="ds_ps", bufs=2, space="PSUM") as dpool, \
                 tc.tile_pool(name="ode_sb", bufs=4) as opool:
                for n in range(steps):
                    def bcol(i):
                        return ct["beta"][:, 6 * n + i - 1 : 6 * n + i]

                    for v in range(NW):
                        vc = bass.ts(v, 1024)
                        zb = zpool.tile([128, 1024], F32, tag="zb")
                        ds = dpool.tile([128, 1024], F32, tag="ds")
                        # t1 = tanh(w + beta_n1)
                        nc.scalar.activation(Tt[:, 0, vc], w[:, vc], AF.Tanh,
                                             bias=bcol(1))
                        # zb = SC*(z + d21 V t1): s2 term then DVE add SC*w
                        mm8(zb, 0, 0, 7, v, True, False)
                        nc.vector.scalar_tensor_tensor(
                            out=zb[:], in0=w[:, vc], scalar=SC, in1=zb[:],
                            op0=MULT, op1=ADD)
                        for i in range(3, 7):
                            nc.scalar.activation(Tt[:, i - 2, vc], zb[:],
                                                 AF.Tanh, bias=bcol(i - 1),
                                                 scale=1.0 / SC)
                            for lidx, a, b in STAGE_MMS[i]:
                                last = (i == 6) and (lidx == 8)
                                mm8(zb, lidx, a, b, v, False, last)
                            if i == 4:  # dS (t1,t3) once t3 exists
                                mm8(ds, 9, 0, 2, v, True, False)
                            if i == 6:  # dS (t4,t5) once t5 exists
                                mm8(ds, 10, 3, 4, v, False, False)
                        nc.scalar.activation(Tt[:, 5, vc], zb[:], AF.Tanh,
                                             bias=bcol(6), scale=1.0 / SC)
                        mm8(ds, 11, 5, 7, v, False, True)
                        # S += ds/DS_SC; dsq = fp8(DS_SC*ds); reuse PSUM tile
                        # for vds = V @ dsq; w += vds/DS_SC
                        nc.vector.scalar_tensor_tensor(
                            out=S0[:, vc], in0=ds[:], scalar=1.0 / DS_SC,
                            in1=S0[:, vc], op0=MULT, op1=ADD)
                        nc.scalar.activation(Tt[:, 6, vc], ds[:], AF.Identity)
                        mm8(ds, 12, 6, 7, v, True, True)
                        nc.vector.scalar_tensor_tensor(
                            out=w[:, vc], in0=ds[:], scalar=1.0 / DS_SC,
                            in1=w[:, vc], op0=MULT, op1=ADD)

                # ---------------- Phase 3: regressor ----------------
                S16 = sreg
                nc.vector.tensor_copy(out=S16[:], in_=S0[:])

                for pr in range(NCH // 2):
                    rp = zpool.tile([128, 1024], F32, tag="zb")
                    for idx in range(2):
                        cc = 2 * pr + idx
                        ccols = bass.ts(cc, 512)
                        orow = slice(64 * idx, 64 * idx + 64)
                        tp_ = (0, 64 * idx)
                        nc.tensor.matmul(rp[orow, 0:512], ct["r1y"][:],
                                         y0[:, ccols], start=True, stop=False,
                                         tile_position=tp_, skip_group_check=True)
                        nc.tensor.matmul(rp[orow, 0:512], ct["r1s"][:],
                                         S16[:, ccols], start=False, stop=True,
                                         tile_position=tp_, skip_group_check=True)
                    rr = opool.tile([128, 512], F16, tag="rr")
                    nc.scalar.activation(rr[:], rp[:, 0:512], AF.Relu,
                                         bias=ct["bias_r"][:])
                    pp = dpool.tile([128, 1024], F32, tag="ds")
                    nc.tensor.matmul(pp[0:4, 0:512], ct["r2"][:], rr[:],
                                     start=True, stop=True,
                                     skip_group_check=True)
                    nc.vector.tensor_scalar_add(out=pred_sb[0:4, bass.ts(pr, 512)],
                                                in0=pp[0:4, 0:512],
                                                scalar1=ct["br2"][0:4])

                # out DMA: pred_sb[32*k, pr, n] -> sample mapping
                pv = pred_sb.rearrange("p (q n) -> p q n", n=512)
                ov = out_t.rearrange("(h q par n) -> h par q n", h=2, par=2, n=512)
                npair = NCH // 2
                # rows 0: (h0, even cc), 32: (h1, even), 64: (h0, odd), 96: (h1, odd)
                for k, (h, par) in enumerate([(0, 0), (1, 0), (0, 1), (1, 1)]):
                    nc.sync.dma_start(
                        out=ov[h, par],
                        in_=pv[k : k + 1, 0:npair, :],
                    )
    nc.compile()
    return nc


_CACHE = {}


def _get_nc(bpc, steps):
    key = (bpc, steps)
    if key not in _CACHE:
        _CACHE[key] = build_nc(bpc, steps)
    return _CACHE[key]


def make_in_maps(inputs):
    x = np.asarray(inputs["x"])
    bpc = x.shape[0] // N_CORES
    x16 = x.reshape(x.shape[0], SEQ * IN_DIM).astype(np.float16)
    # host-side transpose into the conv chunk layout: chunk k holds flat
    # feature rows off(k)..off(k)+127 (s-major (s,c)), samples along free dim
    x16t = np.stack([x16[:, (96 * k if k < 9 else 832):
                          (96 * k if k < 9 else 832) + 128].T
                     for k in range(10)])  # [10, 128, B]
    consts = make_consts(inputs)
    b16, b32, b8 = pack_consts(consts)
    base = {"cb16": b16, "cb32": b32, "cb8": b8}
    return bpc, [dict(base,
                      x16t=np.ascontiguousarray(x16t[:, :, i * bpc:(i + 1) * bpc]))
                 for i in range(N_CORES)]


def kernel(**inputs):
    bpc, in_maps = make_in_maps(inputs)
    nc = _get_nc(bpc, ODE_STEPS)
    res = run_bass_kernel_spmd(nc, in_maps, list(range(N_CORES)))
    return np.concatenate([res.results[i]["out"] for i in range(N_CORES)])



# revision 21
# speedup vs baseline: 3.7551x; 3.5062x over previous
"""Trainium2 Bass kernel for nn_CNN_ODE (CNN encoder + 50-step dopri5 neural ODE + regressor).

Strategy: pure data parallel over 8 NeuronCores (8192 samples/core), parameters
replicated. Per core, activations live feature-on-partition, two batch halves
stacked into 128 partitions ([128, 4096] tiles). The dopri5 step runs in
"z-space" (z = W1 y): every linear combination of stage values is a 64x64
matmul with host-prescaled weights (V = W1@W2) accumulated in PSUM.

ODE matmuls use fp8e4m3 DoubleRow pair-packing: tanh outputs t1..t6 land in
slots of one fp8 tile, and each DoubleRow matmul contracts TWO slots against
two stacked 64x64 weight blocks (contraction K=128x2) at 0.5 cycles/row, so
two chain terms cost one matmul. Per step: 9 chain mms (stages 2..6,
incremental differences), 3 dS mms (scaled identities), and 1 step-update
mm w += V @ ds -- the dS accumulator replaces the 6-term z-chain tail.
Chain weights are scaled by SC=4096 (zb holds SC*z, descaled exactly in the
tanh's ACT scale) and dS identities by 32 to keep fp8 exponents normal; the
per-step +gamma drift of w is folded into per-step tanh bias tables instead
of a DVE op. tanh/silu run on the scalar engine at 128 lanes; encoder and
regressor stay fp16 (validated overall: rel err ~5-9e-3 vs fp32 reference,
dominated by fp8 quantization of the tanh outputs).
"""

import numpy as np

import concourse.bass as bass
import concourse.bacc as bacc
import concourse.mybir as mybir
from concourse.tile import TileContext
from concourse.bass_utils import run_bass_kernel_spmd

F8 = mybir.dt.float8e4
F16 = mybir.dt.float16
F32 = mybir.dt.float32
AF = mybir.ActivationFunctionType

SC = 4096.0  # chain-weight fp8 normalization scale (zb holds SC*z)
DS_SC = 32.0  # dS identity-weight scale (ds PSUM holds DS_SC*ds)

N_CORES = 8
B_TOTAL = 65536
SEQ, IN_DIM, N_KER, KSZ = 40, 24, 36, 3
ENC_DIM, HID, REG = 128, 64, 32
ODE_STEPS = 50
# dopri5 tableau
_A = [
    [1 / 5],
    [3 / 40, 9 / 40],
    [44 / 45, -56 / 15, 32 / 9],
    [19372 / 6561, -25360 / 2187, 64448 / 6561, -212 / 729],
    [9017 / 3168, -355 / 33, 46732 / 5247, 49 / 176, -5103 / 18656],
]
_BW = [35 / 384, 0.0, 500 / 1113, 125 / 192, -2187 / 6784, 11 / 84]


def _ode_coef_lists(dt):
    """Returns (zchain_coefs(15 floats, emission order), ds_coefs(5 floats)).

    Chain covers stages 2..6 only (incremental differences). The step update
    z_{n+1} = z_n + V @ ds + gamma reuses the dS accumulator instead of a
    6-term tail, saving 5 matmul terms per step."""
    coef = np.zeros((7, 7))
    for i in range(2, 7):
        row = _A[i - 2]
        coef[i, 1 : 1 + len(row)] = np.array(row) * dt
    bw = np.array(_BW) * dt
    zc = []
    zc.append(coef[2, 1])
    for i in range(3, 7):
        for j in range(1, i):
            zc.append(coef[i, j] - coef[i - 1, j])
    ds = [bw[j - 1] for j in (1, 3, 4, 5, 6)]
    return zc, ds, coef, bw


def make_consts(inputs, steps=ODE_STEPS):
    """Host-side precompute of all device weight/bias tensors (fp64 math)."""
    f16 = np.float16
    g = {k: np.asarray(v, dtype=np.float64) for k, v in inputs.items() if k != "x"}
    dt = float(g["t_span"][1] - g["t_span"][0]) / steps
    W1, b1 = g["ode1_w"], g["ode1_b"]
    W2, b2 = g["ode2_w"], g["ode2_b"]
    V = W1 @ W2
    cvec = W1 @ b2
    zc, dsc, coef, bw = _ode_coef_lists(dt)

    c = {}
    f8 = mybir.dt.np(F8)
    # ---- ODE pair weights: [128, 13, 2, 128] fp8 block-diagonal DoubleRow
    # matmuls. Emission order (slotA, slotB; slot7 = zeros):
    #  0 s2 (t1,-) | 1 s3 (t1,t2) | 2 s4 (t1,t2) | 3 s4 (t3,-)
    #  4 s5 (t1,t2) | 5 s5 (t3,t4) | 6 s6 (t1,t2) | 7 s6 (t3,t4) | 8 s6 (t5,-)
    #  9 dS (t1,t3) | 10 dS (t4,t5) | 11 dS (t6,-) | 12 vds (dsq,-)
    zcm = [d * V for d in zc]
    pairs = [
        (SC * zcm[0], None), (SC * zcm[1], SC * zcm[2]),
        (SC * zcm[3], SC * zcm[4]), (SC * zcm[5], None),
        (SC * zcm[6], SC * zcm[7]), (SC * zcm[8], SC * zcm[9]),
        (SC * zcm[10], SC * zcm[11]), (SC * zcm[12], SC * zcm[13]),
        (SC * zcm[14], None),
    ]
    E = np.eye(64)
    pairs += [
        (DS_SC * dsc[0] * E, DS_SC * dsc[1] * E),
        (DS_SC * dsc[2] * E, DS_SC * dsc[3] * E),
        (DS_SC * dsc[4] * E, None),
        (V, None),
    ]
    ow8 = np.zeros((128, 13, 2, 128))
    for idx, (a, b) in enumerate(pairs):
        for o, X in ((0, a), (1, b)):
            if X is not None:
                ow8[0:64, idx, o, 0:64] = X.T
                ow8[64:128, idx, o, 64:128] = X.T
    c["ode8_w"] = ow8.astype(f8)
    # per-step biases with gamma-deficit correction: beta_n[i] = beta_i + n*g
    beta = np.zeros((64, 6))
    beta[:, 0] = b1
    for i in range(2, 7):
        beta[:, i - 1] = b1 + coef[i].sum() * cvec
    gam = dt * cvec
    bsteps = beta[:, None, :] + np.arange(steps)[None, :, None] * gam[:, None, None]
    bsteps = bsteps.reshape(64, steps * 6)
    c["beta"] = np.concatenate([bsteps, bsteps], axis=0).astype(np.float32)
    w1bd = np.zeros((128, 128))
    w1bd[0:64, 0:64] = W1.T
    w1bd[64:128, 64:128] = W1.T
    c["w1t"] = w1bd.astype(f16)

    # ---- conv lhsT blocks (c_out padded 36->64)
    cw = g["conv_w"]  # [36, 24, 3]

    def cv_block(n_si, so_count, k_of):
        # rows: (si, ci) over n_si x 24 from row 0; cols: 64*so + co
        out = np.zeros((24 * n_si, 64 * so_count), np.float64)
        for si in range(n_si):
            for ci in range(24):
                for so in range(so_count):
                    k = k_of(si, so)
                    if 0 <= k < 3:
                        out[24 * si + ci, 64 * so : 64 * so + 36] = cw[:, ci, k]
        return out

    # interior pair (4g+1, 4g+2), rhs rows 0..95 (si 0..3): k = si - so
    c["cv_int"] = cv_block(4, 2, lambda si, so: si - so).astype(f16)
    # cross a: rhs rows 0..95 (si<2 pad out as invalid-k): k = si - 2 - so
    c["cv_xa"] = cv_block(4, 2, lambda si, so: si - 2 - so).astype(f16)
    # cross b: chunk g+1 rows 0..47 (si' 0..1): k = si - so + 2
    c["cv_xb"] = cv_block(2, 2, lambda si, so: si - so + 2).astype(f16)
    # edge s0: rows 0..47 (si 0..1): k = si + 1
    c["cv_e0"] = cv_block(2, 1, lambda si, so: si + 1).astype(f16)
    # edge s39 + chunk-9 cross block: chunk 9 is transposed from col 832,
    # so its row u holds flat index 832+u -> s=(832+u)//24, c=(832+u)%24.
    e39 = np.zeros((128, 64))
    xb9 = np.zeros((128, 128))
    for u in range(128):
        s, ci = (832 + u) // 24, (832 + u) % 24
        if s in (38, 39):  # e39: k = s - 38
            e39[u, 0:36] = cw[:, ci, s - 38]
        if s in (36, 37):  # cross-b for pair (35,36): k = (s-36) - so + 2
            for so in range(2):
                k = (s - 36) - so + 2
                if 0 <= k < 3:
                    xb9[u, 64 * so : 64 * so + 36] = cw[:, ci, k]
    c["cv_e39"] = e39.astype(f16)
    c["cv_xb9"] = xb9.astype(f16)
    int9 = np.zeros((128, 128))
    for u in range(128):
        s, ci = (832 + u) // 24, (832 + u) % 24
        for so in range(2):
            k = s - (37 + so) + 1
            if 0 <= k < 3:
                int9[u, 64 * so : 64 * so + 36] = cw[:, ci, k]
    c["cv_int9"] = int9.astype(f16)
    cb = np.zeros((64, 1))
    cb[:36, 0] = g["conv_b"]
    c["conv_bias"] = np.concatenate([cb, cb], axis=0).astype(np.float32)

    # ---- enc1: [128, 20, 128] f16, blocks: 0 = edges(s0 rows0-63, s39 rows64-127),
    # j>=1: s = 2j-1 + r//64, co = r%64 ; flatten index co*40 + s
    e1w = g["enc1_w"]  # [128, 1440]
    e1 = np.zeros((128, 20, 128), np.float64)
    for j in range(20):
        for r in range(128):
            co = r % 64
            if co >= 36:
                continue
            s = (0 if r < 64 else 39) if j == 0 else (2 * j - 1 + r // 64)
            e1[r, j, :] = e1w[:, co * 40 + s]
    c["enc1_w"] = e1.astype(f16)
    c["enc1_bias"] = g["enc1_b"][:, None].astype(np.float32)  # [128,1]
    c["enc2_w"] = g["enc2_w"].T.astype(f16)  # [128, 64]
    c["enc2_bias"] = g["enc2_b"][:, None].astype(np.float32)  # [64,1]

    # ---- regressor
    R1, br1 = g["reg1_w"], g["reg1_b"]
    R2, br2 = g["reg2_w"], g["reg2_b"]
    r1ybd = np.zeros((128, 64))
    r1ybd[0:64, 0:32] = R1.T
    r1ybd[64:128, 32:64] = R1.T
    c["r1y"] = r1ybd.astype(f16)
    r1s = (R1 @ W2).T
    r1sbd = np.zeros((128, 64))
    r1sbd[0:64, 0:32] = r1s
    r1sbd[64:128, 32:64] = r1s
    c["r1s"] = r1sbd.astype(f16)
    bias_r = (R1 @ (steps * dt * b2) + br1)[:, None]
    c["bias_r"] = np.tile(bias_r, (4, 1)).astype(np.float32)  # [128,1]
    r2bd = np.zeros((128, 4))
    for b in range(4):
        r2bd[32 * b : 32 * b + 32, b] = R2[0]
    c["r2"] = r2bd.astype(f16)  # [128,4] block-diagonal
    c["br2"] = np.full((128, 1), br2[0], np.float32)
    return c


def _blob_layout():
    """Pack order + column offsets of consts inside the three dtype blobs."""
    off = {F16: 0, F32: 0, F8: 0}
    lay = {}
    for n, sh, dt in CONST_SPECS:
        cols = int(np.prod(sh[1:]))
        lay[n] = (dt, off[dt], cols, sh)
        off[dt] += cols
    return lay, off[F16], off[F32], off[F8]


def pack_consts(c):
    lay, n16, n32, n8 = _blob_layout()
    b16 = np.zeros((128, n16), np.float16)
    b32 = np.zeros((128, n32), np.float32)
    b8 = np.zeros((128, n8), mybir.dt.np(F8))
    blobs = {F16: b16, F32: b32, F8: b8}
    for n, (dt, off, cols, sh) in lay.items():
        arr = c[n].reshape(sh[0], cols)
        blobs[dt][: sh[0], off : off + cols] = arr
    return b16, b32, b8


CONST_SPECS = [
    ("ode8_w", [128, 13, 2, 128], F8),
    ("beta", [128, ODE_STEPS * 6], F32),
    ("w1t", [128, 128], F16),
    ("cv_int", [96, 128], F16),
    ("cv_xa", [96, 128], F16),
    ("cv_xb", [48, 128], F16),
    ("cv_e0", [48, 64], F16),
    ("cv_e39", [128, 64], F16),
    ("cv_xb9", [128, 128], F16),
    ("cv_int9", [128, 128], F16),
    ("conv_bias", [128, 1], F32),
    ("enc1_w", [128, 20, 128], F16),
    ("enc1_bias", [128, 1], F32),
    ("enc2_w", [128, 64], F16),
    ("enc2_bias", [64, 1], F32),
    ("r1y", [128, 64], F16),
    ("r1s", [128, 64], F16),
    ("bias_r", [128, 1], F32),
    ("r2", [128, 4], F16),
    ("br2", [128, 1], F32),
]


def build_nc(bpc, steps=ODE_STEPS, debug_tap=False):
    """Build the per-core Bass program (SPMD; identical on all cores)."""
    nc = bacc.Bacc("TRN2", target_bir_lowering=False)
    HB = bpc // 2            # stacked tile width (half-batch)
    NCH = HB // 512          # chunk-columns
    NW = HB // 1024          # ODE waves of 1024 cols
    NG = bpc // 512          # encoder groups

    x_in = nc.dram_tensor("x16t", [10, 128, bpc], F16, kind="ExternalInput")
    out_t = nc.dram_tensor("out", [bpc], F32, kind="ExternalOutput")
    dbg_t = (nc.dram_tensor("dbg", [128, bpc // 2], F32, kind="ExternalOutput")
             if debug_tap else None)
    lay, n16, n32, n8 = _blob_layout()
    cb16_in = nc.dram_tensor("cb16", [128, n16], F16, kind="ExternalInput")
    cb32_in = nc.dram_tensor("cb32", [128, n32], F32, kind="ExternalInput")
    cb8_in = nc.dram_tensor("cb8", [128, n8], F8, kind="ExternalInput")

    with TileContext(nc) as tc:
        import contextlib
        es = contextlib.ExitStack()
        with es:
            cpool = es.enter_context(tc.tile_pool(name="consts", bufs=1))
            big = es.enter_context(tc.tile_pool(name="big", bufs=1))

            # const tiles: three packed blobs -> sliced views
            cb16 = cpool.tile([128, n16], F16, tag="cb16", name="cb16")
            cb32 = cpool.tile([128, n32], F32, tag="cb32", name="cb32")
            cb8 = cpool.tile([128, n8], F8, tag="cb8", name="cb8")
            nc.sync.dma_start(out=cb16[:], in_=cb16_in[:])
            nc.sync.dma_start(out=cb32[:], in_=cb32_in[:])
            nc.sync.dma_start(out=cb8[:], in_=cb8_in[:])
            blobs = {F16: cb16, F32: cb32, F8: cb8}
            ct = {}
            for n, (dt, off, cols, sh) in lay.items():
                v = blobs[dt][: sh[0], off : off + cols]
                if len(sh) == 3:
                    v = v.rearrange("p (a b) -> p a b", b=sh[2])
                elif len(sh) == 4:
                    v = v.rearrange("p (a b c) -> p a b c", b=sh[2], c=sh[3])
                ct[n] = v

            # persistent state tiles
            w = big.tile([128, HB], F32, tag="w")
            S0 = big.tile([128, HB], F32, tag="S0")
            y0 = big.tile([128, HB], F16, tag="y0")
            # fp8 slot tile: 0..5 = t1..t6, 6 = dsq, 7 = zeros (pair filler)
            Tt = big.tile([128, 8, HB], F8, tag="Tt")
            sreg = big.tile([128, HB], F16, tag="sreg")
            pred_sb = big.tile([128, HB // 2], F32, tag="pred")
            nc.gpsimd.memset(S0[:], 0.0)
            nc.gpsimd.memset(Tt[:, 7, :], 0.0)

            # ---------------- Phase 1: transpose + encoder ----------------

            def dest_of_group(g):
                # group g (512 samples) -> (row offset, chunk-col) in stacked tiles
                h, cc = (0, g) if g < NG // 2 else (1, g - NG // 2)
                return 64 * h, cc

            with tc.tile_pool(name="enc_sb", bufs=2) as epool, \
                 tc.tile_pool(name="enc_ps", bufs=3, space="PSUM") as cps, \
                 tc.tile_pool(name="enc_ps2", bufs=2, space="PSUM") as eps:
                for g in range(NG):
                    ro, cc = dest_of_group(g)
                    ccols = bass.ts(cc, 512)
                    xt = epool.tile([128, 10, 512], F16, tag="xt")
                    nc.sync.dma_start(
                        out=xt[:],
                        in_=x_in[:, :, g * 512 : (g + 1) * 512].rearrange(
                            "k p n -> p k n"),
                    )
                    h_t = epool.tile([128, 20, 512], F16, tag="h")
                    for pi in range(10):
                        cp = cps.tile([128, 1024], F32, tag="cps")
                        for hf in range(2):
                            b = 2 * pi + hf
                            pc = bass.ts(hf, 512)
                            if b == 0:
                                nc.tensor.matmul(
                                    cp[0:64, pc], ct["cv_e0"][:], xt[0:48, 0, :],
                                    start=True, stop=True, tile_position=(0, 0), skip_group_check=True)
                                nc.tensor.matmul(
                                    cp[64:128, pc], ct["cv_e39"][:], xt[:, 9, :],
                                    start=True, stop=True, tile_position=(0, 64), skip_group_check=True)
                            else:
                                s0 = 2 * b - 1
                                cg, pos = s0 // 4, s0 % 4
                                if pos == 1:
                                    lhs = "cv_int" if cg < 9 else "cv_int9"
                                    rhs = xt[0:96, cg, :] if cg < 9 else xt[:, 9, :]
                                    nc.tensor.matmul(
                                        cp[:, pc], ct[lhs][:], rhs,
                                        start=True, stop=True, skip_group_check=True)
                                else:  # pos == 3, cross
                                    nc.tensor.matmul(
                                        cp[:, pc], ct["cv_xa"][:], xt[0:96, cg, :],
                                        start=True, stop=False, skip_group_check=True)
                                    if cg + 1 < 9:
                                        nc.tensor.matmul(
                                            cp[:, pc], ct["cv_xb"][:],
                                            xt[0:48, cg + 1, :],
                                            start=False, stop=True, skip_group_check=True)
                                    else:
                                        nc.tensor.matmul(
                                            cp[:, pc], ct["cv_xb9"][:],
                                            xt[:, 9, :],
                                            start=False, stop=True, skip_group_check=True)
                        sg = epool.tile([128, 1024], F16, tag="sg")
                        nc.scalar.activation(sg[:], cp[:], AF.Sigmoid,
                                             bias=ct["conv_bias"][:])
                        nc.vector.scalar_tensor_tensor(
                            out=h_t[:, 2 * pi : 2 * pi + 2, :].rearrange(
                                "p a b -> p (a b)"),
                            in0=cp[:], scalar=ct["conv_bias"][:], in1=sg[:],
                            op0=mybir.AluOpType.add, op1=mybir.AluOpType.mult)
                    ep = eps.tile([128, 512], F32, tag="ep")
                    for j in range(20):
                        nc.tensor.matmul(ep[:], ct["enc1_w"][:, j, :], h_t[:, j, :],
                                         start=(j == 0), stop=(j == 19), skip_group_check=True)
                    e1 = epool.tile([128, 512], F16, tag="e1")
                    nc.scalar.activation(e1[:], ep[:], AF.Relu,
                                         bias=ct["enc1_bias"][:])
                    tp = eps.tile([128, 512], F32, tag="ep")
                    nc.tensor.matmul(tp[0:64, :], ct["enc2_w"][:], e1[:],
                                     start=True, stop=True, skip_group_check=True)
                    nc.scalar.activation(y0[ro : ro + 64, ccols], tp[0:64, :],
                                         AF.Identity, bias=ct["enc2_bias"][:])

                # w0 = W1 @ y0 (block-diagonal over sample halves)
                for cc in range(NCH):
                    ccols = bass.ts(cc, 512)
                    wp = eps.tile([128, 512], F32, tag="ep")
                    nc.tensor.matmul(wp[:], ct["w1t"][:], y0[:, ccols],
                                     start=True, stop=True, skip_group_check=True)
                    nc.vector.tensor_copy(out=w[:, ccols], in_=wp[:])

            if dbg_t is not None:
                dbg_sb = big.tile([128, HB], F32, tag="dbgsb")
                nc.vector.tensor_copy(out=dbg_sb[:], in_=y0[:])
                nc.sync.dma_start(out=dbg_t[:], in_=dbg_sb[:])

            # ---------------- Phase 2: ODE (fp8 DoubleRow pairs) ----------
            pstride = Tt.ap[0][0]

            def pair_rhs(a, b, col0, ncols):
                """[128, 2, ncols] fp8 view pairing slots a and b of Tt."""
                return bass.AP(
                    Tt.tensor, Tt.offset + a * HB + col0,
                    [[pstride, 128], [(b - a) * HB, 2], [1, ncols]],
                )

            def mm8(ps, lidx, a, b, v, start, stop):
                for c0 in (0, 512):
                    nc.tensor.matmul(
                        ps[:, c0 : c0 + 512], ct["ode8_w"][:, lidx, :, :],
                        pair_rhs(a, b, v * 1024 + c0, 512),
                        start=start, stop=stop,
                        perf_mode=mybir.MatmulPerfMode.DoubleRow,
                        skip_group_check=True)

            # chain mms per stage i (lidx, slotA, slotB)
            STAGE_MMS = {
                3: [(1, 0, 1)],
                4: [(2, 0, 1), (3, 2, 7)],
                5: [(4, 0, 1), (5, 2, 3)],
                6: [(6, 0, 1), (7, 2, 3), (8, 4, 7)],
            }
            MULT = mybir.AluOpType.mult
            ADD = mybir.AluOpType.add

            with tc.tile_pool(name="ode_ps", bufs=2, space="PSUM") as zpool, \
                 tc.tile_pool(name="ds_ps", bufs=2, space="PSUM") as dpool, \
                 tc.tile_pool(name="ode_sb", bufs=4) as opool:
                for n in range(steps):
                    def bcol(i):
                        return ct["beta"][:, 6 * n + i - 1 : 6 * n + i]

                    for v in range(NW):
                        vc = bass.ts(v, 1024)
                        zb = zpool.tile([128, 1024], F32, tag="zb")
                        ds = dpool.tile([128, 1024], F32, tag="ds")
                        # t1 = tanh(w + beta_n1)
                        nc.scalar.activation(Tt[:, 0, vc], w[:, vc], AF.Tanh,
                                             bias=bcol(1))
                        # zb = SC*(z + d21 V t1): s2 term then DVE add SC*w
                        mm8(zb, 0, 0, 7, v, True, False)
                        nc.vector.scalar_tensor_tensor(
                            out=zb[:], in0=w[:, vc], scalar=SC, in1=zb[:],
                            op0=MULT, op1=ADD)
                        for i in range(3, 7):
                            nc.scalar.activation(Tt[:, i - 2, vc], zb[:],
                                                 AF.Tanh, bias=bcol(i - 1),
                                                 scale=1.0 / SC)
                            for lidx, a, b in STAGE_MMS[i]:
                                last = (i == 6) and (lidx == 8)
                                mm8(zb, lidx, a, b, v, False, last)
                            if i == 4:  # dS (t1,t3) once t3 exists
                                mm8(ds, 9, 0, 2, v, True, False)
                            if i == 6:  # dS (t4,t5) once t5 exists
                                mm8(ds, 10, 3, 4, v, False, False)
                        nc.scalar.activation(Tt[:, 5, vc], zb[:], AF.Tanh,
                                             bias=bcol(6), scale=1.0 / SC)
                        mm8(ds, 11, 5, 7, v, False, True)
                        # S += ds/DS_SC; dsq = fp8(DS_SC*ds); reuse PSUM tile
                        # for vds = V @ dsq; w += vds/DS_SC
                        nc.vector.scalar_tensor_tensor(
                            out=S0[:, vc], in0=ds[:], scalar=1.0 / DS_SC,
                            in1=S0[:, vc], op0=MULT, op1=ADD)
                        nc.scalar.activation(Tt[:, 6, vc], ds[:], AF.Identity)
                        mm8(ds, 12, 6, 7, v, True, True)
                        nc.vector.scalar_tensor_tensor(
                            out=w[:, vc], in0=ds[:], scalar=1.0 / DS_SC,
                            in1=w[:, vc], op0=MULT, op1=ADD)

                # ---------------- Phase 3: regressor ----------------
                S16 = sreg
                nc.vector.tensor_copy(out=S16[:], in_=S0[:])

                for pr in range(NCH // 2):
                    rp = zpool.tile([128, 1024], F32, tag="zb")
                    for idx in range(2):
                        cc = 2 * pr + idx
                        ccols = bass.ts(cc, 512)
                        orow = slice(64 * idx, 64 * idx + 64)
                        tp_ = (0, 64 * idx)
                        nc.tensor.matmul(rp[orow, 0:512], ct["r1y"][:],
                                         y0[:, ccols], start=True, stop=False,
                                         tile_position=tp_, skip_group_check=True)
                        nc.tensor.matmul(rp[orow, 0:512], ct["r1s"][:],
                                         S16[:, ccols], start=False, stop=True,
                                         tile_position=tp_, skip_group_check=True)
                    rr = opool.tile([128, 512], F16, tag="rr")
                    nc.scalar.activation(rr[:], rp[:, 0:512], AF.Relu,
                                         bias=ct["bias_r"][:])
                    pp = dpool.tile([128, 1024], F32, tag="ds")
                    nc.tensor.matmul(pp[0:4, 0:512], ct["r2"][:], rr[:],
                                     start=True, stop=True,
                                     skip_group_check=True)
                    nc.vector.tensor_scalar_add(out=pred_sb[0:4, bass.ts(pr, 512)],
                                                in0=pp[0:4, 0:512],
                                                scalar1=ct["br2"][0:4])

                # out DMA: pred_sb[32*k, pr, n] -> sample mapping
                pv = pred_sb.rearrange("p (q n) -> p q n", n=512)
                ov = out_t.rearrange("(h q par n) -> h par q n", h=2, par=2, n=512)
                npair = NCH // 2
                # rows 0: (h0, even cc), 32: (h1, even), 64: (h0, odd), 96: (h1, odd)
                for k, (h, par) in enumerate([(0, 0), (1, 0), (0, 1), (1, 1)]):
                    nc.sync.dma_start(
                        out=ov[h, par],
                        in_=pv[k : k + 1, 0:npair, :],
                    )
    nc.compile()
    return nc


_CACHE = {}


def _get_nc(bpc, steps):
    key = (bpc, steps)
    if key not in _CACHE:
        _CACHE[key] = build_nc(bpc, steps)
    return _CACHE[key]


def make_in_maps(inputs):
    x = np.asarray(inputs["x"])
    bpc = x.shape[0] // N_CORES
    x16 = x.reshape(x.shape[0], SEQ * IN_DIM).astype(np.float16)
    # host-side transpose into the conv chunk layout: chunk k holds flat
    # feature rows off(k)..off(k)+127 (s-major (s,c)), samples along free dim
    x16t = np.stack([x16[:, (96 * k if k < 9 else 832):
                          (96 * k if k < 9 else 832) + 128].T
                     for k in range(10)])  # [10, 128, B]
    consts = make_consts(inputs)
    b16, b32, b8 = pack_consts(consts)
    base = {"cb16": b16, "cb32": b32, "cb8": b8}
    return bpc, [dict(base,
                      x16t=np.ascontiguousarray(x16t[:, :, i * bpc:(i + 1) * bpc]))
                 for i in range(N_CORES)]


def kernel(**inputs):
    bpc, in_maps = make_in_maps(inputs)
    nc = _get_nc(bpc, ODE_STEPS)
    res = run_bass_kernel_spmd(nc, in_maps, list(range(N_CORES)))
    return np.concatenate([res.results[i]["out"] for i in range(N_CORES)])

